# revision 12
# baseline (speedup 1.0000x reference)
"""GNN message passing (gather + weighted segment-sum) on 8 Trainium2 cores.

out[n, :] = sum_{e : dst[e] == n} weight[e] * queue[src[e], :]

Strategy
--------
Edges are sharded by destination window (128 destination nodes per window,
49 windows per core).  Each core:
  * gathers queue[src] rows straight from HBM with `dma_gather`
    (indices are int16, so the 50000-row queue is addressed as two
    parity-interleaved 25000-row strided views: even rows / odd rows),
  * builds a weighted one-hot matrix H[e, j] = weight[e] * (dstoff[e] == j)
    per 128-edge block with a single dual-op tensor_scalar on the DVE,
  * accumulates H.T @ G into a [128, 64] PSUM tile per window on the
    TensorEngine (fp32),
  * copies each finished window to SBUF and DMAs it to its slice of the
    output.

All data-dependent structure (edges per window, padded uniformly across
cores so one SPMD NEFF serves all 8 cores) is computed on the host from the
actual inputs at call time.
"""

import contextlib
import sys

sys.path.insert(0, "/opt/trn_rl_repo")

import numpy as np

import concourse.bass as bass  # noqa: F401
import concourse.mybir as mybir
import concourse.tile as tile
from concourse import bacc
from concourse.bass_utils import run_bass_kernel_spmd

P = 128
N_CORES = 8

N_NODES = 50000
N_EDGES = 800000
D_FEAT = 64


def _plan(n_nodes, n_cores):
    """Windows-per-core and chunking. All cores run the identical program."""
    n_windows = -(-n_nodes // P)
    wpc = -(-n_windows // n_cores)
    # chunk width: largest divisor of wpc that keeps gather tiles a sane size
    cw = max(d for d in range(1, min(wpc, 8) + 1) if wpc % d == 0)
    nchunk = wpc // cw
    return wpc, cw, nchunk


def _host_prep(weight, src, dst, n_nodes, wpc, cw, nchunk, n_cores):
    """Bucket edges by (core, window, src parity); pad uniformly.

    Returns (epw, idx_hbm, aux_hbm):
      idx_hbm [n_cores, nchunk, 2, 128, cw*epw//16] int16  (dma_gather layout)
      aux_hbm [n_cores, nchunk, 128, 4*cw*nb] f32  (dstoff then weight, packed
              so block k of half h of window j sits at column (h*cw+j)*nb+k)
    """
    e = src.shape[0]
    src = np.asarray(src).astype(np.int64).reshape(-1)
    dst = np.asarray(dst).astype(np.int64).reshape(-1)
    wgt = np.asarray(weight, dtype=np.float32).reshape(-1)

    w = dst >> 7
    core = w // wpc
    lw = w - core * wpc
    half = src & 1
    hidx = (src >> 1).astype(np.int16)
    dstoff = (dst & 127).astype(np.float32)

    nbuckets = n_cores * wpc * 2
    key = (core * wpc + lw) * 2 + half
    order = np.argsort(key, kind="stable")
    counts = np.bincount(key, minlength=nbuckets)
    epw = int(-(-max(int(counts.max()), 1) // P) * P)
    offs = np.zeros(nbuckets + 1, np.int64)
    np.cumsum(counts, out=offs[1:])
    skey = key[order]
    rank = np.arange(e, dtype=np.int64) - offs[skey]
    dest = skey * epw + rank

    idx_arr = np.zeros(nbuckets * epw, np.int16)
    dst_arr = np.zeros(nbuckets * epw, np.float32)
    wgt_arr = np.zeros(nbuckets * epw, np.float32)
    idx_arr[dest] = hidx[order]
    dst_arr[dest] = dstoff[order]
    wgt_arr[dest] = wgt[order]

    nb = epw // P
    big = cw * epw  # indices per gather call
    shp = (n_cores, nchunk, cw, 2, epw)
    idx_arr = idx_arr.reshape(shp)
    dst_arr = dst_arr.reshape(shp)
    wgt_arr = wgt_arr.reshape(shp)

    # idx: window-major edge list per (core, chunk, half), wrapped mod 16 and
    # replicated to 128 partitions (8 Q7 cores each read a 16-partition copy).
    a = idx_arr.transpose(0, 1, 3, 2, 4).reshape(n_cores, nchunk, 2, big // 16, 16)
    a = a.transpose(0, 1, 2, 4, 3)  # [.., 16, big//16]
    idx_hbm = np.broadcast_to(
        a[:, :, :, None, :, :], (n_cores, nchunk, 2, 8, 16, big // 16)
    ).reshape(n_cores, nchunk, 2, P, big // 16)
    idx_hbm = np.ascontiguousarray(idx_hbm)

    def pack(x):
        y = x.reshape(n_cores, nchunk, cw, 2, nb, P)
        y = y.transpose(0, 1, 5, 3, 2, 4)  # [core, chunk, P, h, j, k]
        return y.reshape(n_cores, nchunk, P, 2 * cw * nb)

    aux_hbm = np.concatenate([pack(dst_arr), pack(wgt_arr)], axis=3)
    aux_hbm = np.ascontiguousarray(aux_hbm)
    return epw, idx_hbm, aux_hbm


def _build(n_nodes, d, epw, wpc, cw, nchunk, iters=1):
    f32 = mybir.dt.float32
    nb = epw // P
    big = cw * epw
    bpc = cw * nb  # blocks per half per chunk
    ne = n_nodes // 2
    assert n_nodes % 2 == 0

    nc = bacc.Bacc("TRN2", target_bir_lowering=False, debug=False)

    queue_t = nc.dram_tensor("queue", [n_nodes, d], f32, kind="ExternalInput")
    idx_t = nc.dram_tensor(
        "idx", [nchunk, 2, P, big // 16], mybir.dt.int16, kind="ExternalInput"
    )
    aux_t = nc.dram_tensor("aux", [nchunk, P, 4 * bpc], f32, kind="ExternalInput")
    iota_t = nc.dram_tensor("iota", [P, P], f32, kind="ExternalInput")
    out_t = nc.dram_tensor("out", [wpc * P, d], f32, kind="ExternalOutput")

    q2 = queue_t.ap().rearrange("(n t) d -> n (t d)", t=2)  # [ne, 2d]
    qviews = [q2[:, 0:d], q2[:, d : 2 * d]]
    assert qviews[0].shape == (ne, d)

    with tile.TileContext(nc) as tc:
        with (
            tc.tile_pool(name="const", bufs=1) as cpool,
            tc.tile_pool(name="io", bufs=2) as iopool,
            tc.tile_pool(name="gat", bufs=2) as gpool,
            tc.tile_pool(name="hot", bufs=4) as hpool,
            tc.tile_pool(name="ost", bufs=4) as opool,
            tc.tile_pool(name="ps", bufs=4, space="PSUM") as ppool,
        ):
            iota_f = cpool.tile([P, P], f32)
            nc.sync.dma_start(out=iota_f[:], in_=iota_t.ap()[:, :])

            loop = tc.For_i(0, iters, 1) if iters > 1 else contextlib.nullcontext()
            with loop:
                for c in range(nchunk):
                    idxs = []
                    for h in (0, 1):
                        it = iopool.tile(
                            [P, big // 16], mybir.dt.int16, tag=f"idx{h}"
                        )
                        nc.sync.dma_start(out=it[:], in_=idx_t.ap()[c, h])
                        idxs.append(it)
                    aux = iopool.tile([P, 4 * bpc], f32, tag="aux")
                    nc.sync.dma_start(out=aux[:], in_=aux_t.ap()[c])

                    gt = []
                    for h in (0, 1):
                        g = gpool.tile([P, bpc, d], f32, tag=f"g{h}")
                        nc.gpsimd.dma_gather(
                            out_ap=g[:],
                            in_ap=qviews[h],
                            idxs_ap=idxs[h][:],
                            num_idxs=big,
                            num_idxs_reg=big,
                            elem_size=d,
                            elem_step=2 * d,
                            single_packet=False,
                        )
                        gt.append(g)

                    for j in range(cw):
                        ps = ppool.tile([P, d], f32)
                        for h in (0, 1):
                            for k in range(nb):
                                col = (h * cw + j) * nb + k
                                gi = j * nb + k
                                hot = hpool.tile([P, P], f32, tag="hot")
                                nc.vector.tensor_scalar(
                                    hot[:],
                                    iota_f[:],
                                    aux[:, col : col + 1],
                                    aux[:, 2 * bpc + col : 2 * bpc + col + 1],
                                    mybir.AluOpType.is_equal,
                                    mybir.AluOpType.mult,
                                )
                                nc.tensor.matmul(
                                    ps[:],
                                    lhsT=hot[:],
                                    rhs=gt[h][:, gi, :],
                                    start=(h == 0 and k == 0),
                                    stop=(h == 1 and k == nb - 1),
                                )
                        ot = opool.tile([P, d], f32, tag="ot")
                        nc.scalar.copy(ot[:], ps[:])
                        wg = c * cw + j
                        nc.sync.dma_start(
                            out=out_t.ap()[wg * P : (wg + 1) * P, :], in_=ot[:]
                        )
    nc.compile()
    return nc


def _make_inputs(queue, idx_hbm, aux_hbm, n_cores):
    iota_np = np.ascontiguousarray(
        np.broadcast_to(np.arange(P, dtype=np.float32), (P, P))
    )
    return [
        {"queue": queue, "idx": idx_hbm[c], "aux": aux_hbm[c], "iota": iota_np}
        for c in range(n_cores)
    ]


def _run(queue, weight, src, dst, n_nodes, d, n_cores, trace=False, iters=1):
    queue = np.ascontiguousarray(np.asarray(queue, dtype=np.float32))
    wpc, cw, nchunk = _plan(n_nodes, n_cores)
    epw, idx_hbm, aux_hbm = _host_prep(
        weight, src, dst, n_nodes, wpc, cw, nchunk, n_cores
    )
    nc = _build(n_nodes, d, epw, wpc, cw, nchunk, iters=iters)
    in_maps = _make_inputs(queue, idx_hbm, aux_hbm, n_cores)
    res = run_bass_kernel_spmd(nc, in_maps, core_ids=list(range(n_cores)), trace=trace)
    full = np.concatenate([res.results[c]["out"] for c in range(n_cores)], axis=0)
    return full[:n_nodes], res


def kernel(queue, weight, src, dst):
    out, _ = _run(queue, weight, src, dst, N_NODES, D_FEAT, N_CORES)
    return out
